# revision 3
# baseline (speedup 1.0000x reference)
"""Self-contained 8-core Trainium2 Bass kernel for multi-head attention.

Problem: B=4, S=2048, E=1024, H=16, D=64 MHA with key-position mask.
Sharding: 8 cores = 4 batches x 2 head-groups (8 heads / 512 feats each).
Each core computes QKV projections for its (batch, head-group), attention,
and a partial output projection  attn_cat @ Wo[group_rows, :].  The host
sums the two partial outputs per batch; bo is folded into group-0 cores.

Key tricks:
  - mask folded into V:  V_aug = [mask*V | mask] per head, so the softmax
    mask AND the denominator (row 64 of the attnV PSUM) are free.
  - exp has no max-subtraction (scores ~ N(0,1), no overflow) and the
    1/sqrt(D) scale is folded into the ACT instruction's scale field.
  - scores^T layout [k, q] so the attnV matmul needs no transposes;
    head pairs run on array row-halves (K=64 each) concurrently.
"""

import os
import numpy as np

B = 4
S = 2048
E = 1024
H = 16
D = 64
G = 2                 # head groups (tensor parallel)
HL = H // G           # heads per core
FL = HL * D           # local features = 512
P = 128
NCORES = 8

_NC_CACHE = {}
LAST_RESULTS = None


def build_nc(s=S):
    """Build (and cache) the single-core Bass module, SPMD across 8 cores."""
    if s in _NC_CACHE:
        return _NC_CACHE[s]

    from contextlib import ExitStack

    import concourse.mybir as mybir
    import concourse.tile as tile
    from concourse import bacc

    f32 = mybir.dt.float32
    bf16 = mybir.dt.bfloat16
    EXP = mybir.ActivationFunctionType.Exp

    EC = E // P           # 8 e-chunks
    FC = FL // P          # 4 local feature chunks
    n_st = s // 512       # seq tiles for projections
    n_kb = s // P         # key chunks
    QW = min(s, 1024)     # q tile width in attention
    n_qt = s // QW
    n_qb = s // P         # out-proj q blocks

    nc = bacc.Bacc("TRN2", target_bir_lowering=False, debug=False,
                   num_devices=NCORES)

    xq_d = nc.dram_tensor("xq", [s, E], f32, kind="ExternalInput").ap()
    xk_d = nc.dram_tensor("xk", [s, E], f32, kind="ExternalInput").ap()
    xv_d = nc.dram_tensor("xv", [s, E], f32, kind="ExternalInput").ap()
    wq_d = nc.dram_tensor("wq", [E, FL], f32, kind="ExternalInput").ap()
    wk_d = nc.dram_tensor("wk", [E, FL], f32, kind="ExternalInput").ap()
    wv_d = nc.dram_tensor("wv", [E, FL], f32, kind="ExternalInput").ap()
    wo_d = nc.dram_tensor("wo", [FL, E], f32, kind="ExternalInput").ap()
    bq_d = nc.dram_tensor("bq", [FL], f32, kind="ExternalInput").ap()
    bk_d = nc.dram_tensor("bk", [FL], f32, kind="ExternalInput").ap()
    bv_d = nc.dram_tensor("bv", [FL], f32, kind="ExternalInput").ap()
    bo_d = nc.dram_tensor("bo", [E], f32, kind="ExternalInput").ap()
    mask_d = nc.dram_tensor("maskf", [s], f32, kind="ExternalInput").ap()
    out_d = nc.dram_tensor("out", [s, E], f32, kind="ExternalOutput").ap()

    with tile.TileContext(nc) as tc, ExitStack() as ctx:
        consts = ctx.enter_context(tc.tile_pool(name="consts", bufs=1))
        persist = ctx.enter_context(tc.tile_pool(name="persist", bufs=1))
        stage = ctx.enter_context(tc.tile_pool(name="stage", bufs=3))

        # ---------- constants: weights (cast to bf16), biases, mask ----------
        wq_sb = consts.tile([P, EC, FL], bf16, tag="wq")
        wk_sb = consts.tile([P, EC, FL], bf16, tag="wk")
        wv_sb = consts.tile([P, EC, FL], bf16, tag="wv")
        for w_sb, w_d in ((wq_sb, wq_d), (wk_sb, wk_d), (wv_sb, wv_d)):
            for ec in range(EC):
                t = stage.tile([P, E], f32, tag="xa")
                nc.scalar.dma_start(t[:, :FL], w_d[ec * P:(ec + 1) * P, :])
                nc.vector.tensor_copy(w_sb[:, ec, :], t[:, :FL])
        wo_sb = consts.tile([P, FC, E], bf16, tag="wo")
        for fc in range(FC):
            t = stage.tile([P, E], f32, tag="xa")
            nc.scalar.dma_start(t, wo_d[fc * P:(fc + 1) * P, :])
            nc.vector.tensor_copy(wo_sb[:, fc, :], t)

        bq_sb = consts.tile([P, FC], f32, tag="bq")
        bk_sb = consts.tile([P, FC], f32, tag="bk")
        nc.sync.dma_start(bq_sb, bq_d.rearrange("(c p) -> p c", p=P))
        nc.sync.dma_start(bk_sb, bk_d.rearrange("(c p) -> p c", p=P))

        bv_row = consts.tile([1, FL], f32, tag="bv_row")
        nc.sync.dma_start(bv_row, bv_d[None, :])
        bv_bc = consts.tile([P, FL], f32, tag="bv_bc")
        nc.gpsimd.partition_broadcast(bv_bc, bv_row)

        bo_row = consts.tile([1, E], f32, tag="bo_row")
        nc.sync.dma_start(bo_row, bo_d[None, :])
        bo_bc = consts.tile([P, E], f32, tag="bo_bc")
        nc.gpsimd.partition_broadcast(bo_bc, bo_row)

        maskc = consts.tile([P, n_kb], f32, tag="maskc")
        nc.sync.dma_start(maskc, mask_d.rearrange("(c p) -> p c", p=P))
        maskc_bf = consts.tile([P, n_kb], bf16, tag="maskc_bf")
        nc.vector.tensor_copy(maskc_bf, maskc)

        # ---------- persistent slabs ----------
        QT = persist.tile([P, FC, s], bf16, tag="QT")     # Q^T [feat, s]
        KT = persist.tile([P, FC, s], bf16, tag="KT")     # K^T [feat, s]
        Vaug = persist.tile([P, n_kb, HL * (D + 1)], bf16, tag="Vaug")
        AC = persist.tile([P, FC, s], bf16, tag="AC")     # attn_cat^T

        # ---------- phase A+B: transpose inputs + QKV projections ----------
        with tc.tile_pool(name="xtp", bufs=3) as xtp, \
             tc.tile_pool(name="bpsum", bufs=2, space="PSUM") as bpsum:
            for st in range(n_st):
                xts = {}
                for tname, xd in (("q", xq_d), ("k", xk_d), ("v", xv_d)):
                    xt = xtp.tile([P, EC, 512], bf16, tag="xt")
                    for sb in range(4):
                        row0 = st * 512 + sb * P
                        xa = stage.tile([P, E], f32, tag="xa")
                        nc.scalar.dma_start(xa, xd[row0:row0 + P, :])
                        xb = stage.tile([P, E], bf16, tag="xb")
                        nc.vector.tensor_copy(xb, xa)
                        for ec in range(EC):
                            nc.sync.dma_start_transpose(
                                xt[:, ec, sb * P:(sb + 1) * P],
                                xb[:, ec * P:(ec + 1) * P])
                    xts[tname] = xt

                for xt, w_sb, b_sb, OUT in ((xts["q"], wq_sb, bq_sb, QT),
                                            (xts["k"], wk_sb, bk_sb, KT)):
                    for fc in range(FC):
                        ps = bpsum.tile([P, 512], f32, tag="bps")
                        for ec in range(EC):
                            nc.tensor.matmul(
                                ps, lhsT=w_sb[:, ec, fc * P:(fc + 1) * P],
                                rhs=xt[:, ec, :],
                                start=(ec == 0), stop=(ec == EC - 1))
                        nc.vector.tensor_scalar_add(
                            OUT[:, fc, st * 512:(st + 1) * 512], ps,
                            b_sb[:, fc:fc + 1])

                for sb in range(4):
                    kb = st * 4 + sb
                    ps = bpsum.tile([P, 512], f32, tag="bps")
                    for ec in range(EC):
                        nc.tensor.matmul(
                            ps, lhsT=xts["v"][:, ec, sb * P:(sb + 1) * P],
                            rhs=wv_sb[:, ec, :],
                            start=(ec == 0), stop=(ec == EC - 1))
                    vrow = Vaug[:, kb, :].rearrange("p (h c) -> p h c", c=D + 1)
                    nc.vector.tensor_add(
                        vrow[:, :, 0:D],
                        ps.rearrange("p (h d) -> p h d", d=D),
                        bv_bc.rearrange("p (h d) -> p h d", d=D))
                    nc.vector.tensor_scalar_mul(
                        vrow[:, :, 0:D], vrow[:, :, 0:D], maskc[:, kb:kb + 1])
                    nc.vector.tensor_copy(
                        vrow[:, :, D:D + 1],
                        maskc_bf[:, kb:kb + 1, None].to_broadcast([P, HL, 1]))

        # ---------- phase C: attention ----------
        with tc.tile_pool(name="spsum", bufs=2, space="PSUM") as spsum, \
             tc.tile_pool(name="apsum", bufs=2, space="PSUM") as apsum, \
             tc.tile_pool(name="epool", bufs=8) as epool, \
             tc.tile_pool(name="npool", bufs=2) as npool:
            for qt in range(n_qt):
                q0 = qt * QW
                for pr in range(HL // 2):
                    a_ps = [apsum.tile([D + 1, QW], f32, tag="aps",
                                       name=f"aps{j}")
                            for j in range(2)]
                    for kb in range(n_kb):
                        for j in range(2):
                            h = 2 * pr + j
                            base = j * 64
                            sp = spsum.tile([P, QW], f32, tag="sps")
                            lhsT = KT[base:base + 64, pr, kb * P:(kb + 1) * P]
                            rhs = QT[base:base + 64, pr, q0:q0 + QW]
                            for hf in range(QW // 512):
                                nc.tensor.matmul(
                                    sp[:, hf * 512:(hf + 1) * 512], lhsT=lhsT,
                                    rhs=rhs[:, hf * 512:(hf + 1) * 512],
                                    start=True, stop=True)
                            e = epool.tile([P, QW], bf16, tag="e")
                            nc.scalar.activation(e, sp, EXP, scale=0.125)
                            lv = Vaug[:, kb, h * (D + 1):(h + 1) * (D + 1)]
                            for hf in range(QW // 512):
                                nc.tensor.matmul(
                                    a_ps[j][:, hf * 512:(hf + 1) * 512],
                                    lhsT=lv,
                                    rhs=e[:, hf * 512:(hf + 1) * 512],
                                    start=(kb == 0), stop=(kb == n_kb - 1))
                    for j in range(2):
                        rec = npool.tile([1, QW], f32, tag="rec")
                        nc.vector.reciprocal(rec, a_ps[j][D:D + 1, :])
                        rb = npool.tile([64, QW], f32, tag="rb")
                        nc.gpsimd.partition_broadcast(rb, rec)
                        nc.vector.tensor_mul(
                            AC[j * 64:(j + 1) * 64, pr, q0:q0 + QW],
                            a_ps[j][0:D, :], rb)

        # ---------- phase D: output projection ----------
        with tc.tile_pool(name="dpsum", bufs=2, space="PSUM") as dpsum, \
             tc.tile_pool(name="dout", bufs=3) as dout:
            for qb in range(n_qb):
                for et in range(E // 512):
                    ps = dpsum.tile([P, 512], f32, tag="dps")
                    for fc in range(FC):
                        nc.tensor.matmul(
                            ps, lhsT=AC[:, fc, qb * P:(qb + 1) * P],
                            rhs=wo_sb[:, fc, et * 512:(et + 1) * 512],
                            start=(fc == 0), stop=(fc == FC - 1))
                    o = dout.tile([P, 512], f32, tag="o")
                    nc.vector.tensor_add(o, ps, bo_bc[:, et * 512:(et + 1) * 512])
                    nc.sync.dma_start(
                        out_d[qb * P:(qb + 1) * P, et * 512:(et + 1) * 512], o)

    nc.compile()
    _NC_CACHE[s] = nc
    return nc


def make_in_maps(query, key, value, mask, Wq, bq, Wk, bk, Wv, bv, Wo, bo, s=S):
    """Shard full inputs into the 8 per-core input maps."""
    query = np.asarray(query, np.float32)
    key = np.asarray(key, np.float32)
    value = np.asarray(value, np.float32)
    Wq = np.asarray(Wq, np.float32)
    Wk = np.asarray(Wk, np.float32)
    Wv = np.asarray(Wv, np.float32)
    Wo = np.asarray(Wo, np.float32)
    bq = np.asarray(bq, np.float32)
    bk = np.asarray(bk, np.float32)
    bv = np.asarray(bv, np.float32)
    bo = np.asarray(bo, np.float32)
    maskf = np.asarray(mask).reshape(B, -1).astype(np.float32)  # (B, S)

    zeros_bo = np.zeros_like(bo)
    in_maps = []
    for c in range(NCORES):
        b, g = divmod(c, G)
        fs = slice(g * FL, (g + 1) * FL)
        in_maps.append({
            "xq": np.ascontiguousarray(query[b, :s]),
            "xk": np.ascontiguousarray(key[b, :s]),
            "xv": np.ascontiguousarray(value[b, :s]),
            "wq": np.ascontiguousarray(Wq[:, fs]),
            "wk": np.ascontiguousarray(Wk[:, fs]),
            "wv": np.ascontiguousarray(Wv[:, fs]),
            "wo": np.ascontiguousarray(Wo[fs, :]),
            "bq": np.ascontiguousarray(bq[fs]),
            "bk": np.ascontiguousarray(bk[fs]),
            "bv": np.ascontiguousarray(bv[fs]),
            "bo": bo if g == 0 else zeros_bo,
            "maskf": np.ascontiguousarray(maskf[b, :s]),
        })
    return in_maps


def kernel(query, key, value, mask, Wq, bq, Wk, bk, Wv, bv, Wo, bo):
    global LAST_RESULTS
    from concourse import bass_utils

    nc = build_nc(S)
    in_maps = make_in_maps(query, key, value, mask,
                           Wq, bq, Wk, bk, Wv, bv, Wo, bo)
    trace = bool(int(os.environ.get("KTRACE", "0")))
    if trace:
        try:
            from antenv.axon_hooks import get_axon_ntff_profile_hook  # noqa: F401
        except ImportError:
            trace = False
    res = bass_utils.run_bass_kernel_spmd(
        nc, in_maps, core_ids=list(range(NCORES)), trace=trace)
    LAST_RESULTS = res

    out = np.zeros((B, S, E), np.float32)
    for c in range(NCORES):
        b, _ = divmod(c, G)
        out[b] += res.results[c]["out"]
    return out
